# revision 4
# baseline (speedup 1.0000x reference)
"""Trainium2 Bass kernel: 3x3 valid 2D cross-correlation on an 8192x8192 f32 image.

Strategy (8 NeuronCores, pure spatial/data parallel):
  - Row-shard: core i receives input rows [1024*i, 1024*i + 1026) (the 2-row
    halo comes for free since we shard on the host from the full input; the
    tail cores' out-of-range rows are zero-padded and the corresponding
    output rows discarded at gather).
  - Per core: for each 128-input-row tile, the conv is computed as 3
    TensorEngine matmuls accumulating into PSUM:
        out[y, c] = sum_dx (M_dx.T @ X)[y, c+dx]
    where M_dx[k, y] = w[k-y, dx] is a 3-diagonal band matrix built on the
    host from the 3x3 weight. Data moves through the PE in float32r mode
    (full fp32 bits, fast 4-byte streaming path, ~1 cycle/column).
  - PSUM chunks (512 cols) are evacuated to SBUF by the Vector/Scalar
    engines (alternating), then DMA'd to DRAM.
"""

import numpy as np

import concourse.mybir as mybir
from concourse import bacc
from concourse.tile import TileContext
from concourse.bass_utils import run_bass_kernel_spmd

H = W = 8192
KH = KW = 3
N_CORES = 8
OUT_H = H - KH + 1  # 8190
OUT_W = W - KW + 1  # 8190

ROWS_PER_CORE = 1024          # output rows computed per core (core 7: keep 1022)
IN_ROWS_PER_CORE = ROWS_PER_CORE + KH - 1  # 1026
TILE_OUT = 126                # output rows per 128-partition input tile
CHUNK = 512                   # PSUM bank width (fp32)

_NC_CACHE = {}


def _build_program():
    """Build the per-core Bass program (identical on all 8 cores)."""
    nc = bacc.Bacc("TRN2", target_bir_lowering=False, debug=False)
    x = nc.declare_dram_parameter(
        "x", [IN_ROWS_PER_CORE, W], mybir.dt.float32r, isOutput=False
    )
    m = nc.declare_dram_parameter(
        "m", [128, 3 * TILE_OUT], mybir.dt.float32r, isOutput=False
    )
    y = nc.declare_dram_parameter(
        "y", [ROWS_PER_CORE, OUT_W], mybir.dt.float32, isOutput=True
    )

    n_tiles = -(-ROWS_PER_CORE // TILE_OUT)  # 9 (8 full + 1 of 16 rows)
    n_chunks = -(-OUT_W // CHUNK)            # 16 (15 full + 1 of 510)

    STORE_GROUP = 4  # chunks per output DMA (~1 MiB each)

    with TileContext(nc) as tc:
        with (
            tc.tile_pool(name="mp", bufs=1) as mpool,
            tc.tile_pool(name="xp", bufs=3) as xpool,
            tc.tile_pool(name="op", bufs=2) as opool,
            tc.tile_pool(name="pp", bufs=8, space="PSUM") as ppool,
        ):
            # first x tile load leads the program so DMA starts immediately
            xts = [None] * n_tiles
            xts[0] = xpool.tile([128, W], mybir.dt.float32r, name="xt", tag="xt")
            nc.sync.dma_start(out=xts[0][:128], in_=x[0:128, :])

            mt = mpool.tile([128, 3 * TILE_OUT], mybir.dt.float32r)
            nc.sync.dma_start(out=mt[:], in_=m[:])

            for t in range(n_tiles):
                r0 = t * TILE_OUT
                rows_out = min(TILE_OUT, ROWS_PER_CORE - r0)
                rows_in = rows_out + KH - 1

                if xts[t] is None:
                    xts[t] = xpool.tile([128, W], mybir.dt.float32r, name="xt", tag="xt")
                    nc.sync.dma_start(out=xts[t][:rows_in],
                                      in_=x[r0:r0 + rows_in, :])
                xt = xts[t]

                ot = opool.tile([128, OUT_W], mybir.dt.float32)
                for c in range(n_chunks):
                    c0 = c * CHUNK
                    wid = min(CHUNK, OUT_W - c0)
                    pt = ppool.tile([128, CHUNK], mybir.dt.float32)
                    for dx in range(KW):
                        nc.tensor.matmul(
                            pt[:rows_out, :wid],
                            mt[:rows_in, dx * TILE_OUT:dx * TILE_OUT + rows_out],
                            xt[:rows_in, c0 + dx:c0 + dx + wid],
                            start=(dx == 0),
                            stop=(dx == KW - 1),
                        )
                    if c % 2 == 0:
                        nc.scalar.copy(out=ot[:rows_out, c0:c0 + wid],
                                       in_=pt[:rows_out, :wid])
                    else:
                        nc.vector.tensor_copy(out=ot[:rows_out, c0:c0 + wid],
                                              in_=pt[:rows_out, :wid])
                    # store each group of chunks as soon as it is copied
                    if c % STORE_GROUP == STORE_GROUP - 1 or c == n_chunks - 1:
                        g0 = (c - c % STORE_GROUP) * CHUNK
                        g1 = c0 + wid
                        nc.scalar.dma_start(out=y[r0:r0 + rows_out, g0:g1],
                                            in_=ot[:rows_out, g0:g1])
    nc.compile()
    return nc


def _get_program():
    if "nc" not in _NC_CACHE:
        _NC_CACHE["nc"] = _build_program()
    return _NC_CACHE["nc"]


def _band_matrices(weight: np.ndarray) -> np.ndarray:
    """m[k, dx*126 + y] = w[k-y, dx] for 0 <= k-y < 3."""
    m = np.zeros((128, 3 * TILE_OUT), dtype=np.float32)
    for dx in range(KW):
        for dy in range(KH):
            ys = np.arange(TILE_OUT)
            m[ys + dy, dx * TILE_OUT + ys] = weight[dy, dx]
    return m


def kernel(x: np.ndarray, weight: np.ndarray) -> np.ndarray:
    x = np.ascontiguousarray(np.asarray(x, dtype=np.float32))
    weight = np.asarray(weight, dtype=np.float32)
    assert x.shape == (H, W) and weight.shape == (KH, KW)

    m = _band_matrices(weight)

    # shard rows with halo; zero-pad past the bottom edge
    in_maps = []
    for i in range(N_CORES):
        r0 = i * ROWS_PER_CORE
        r1 = min(r0 + IN_ROWS_PER_CORE, H)
        shard = np.zeros((IN_ROWS_PER_CORE, W), dtype=np.float32)
        shard[: r1 - r0] = x[r0:r1]
        in_maps.append({"x": shard, "m": m})

    nc = _get_program()
    res = run_bass_kernel_spmd(nc, in_maps, core_ids=list(range(N_CORES)))

    out = np.empty((OUT_H, OUT_W), dtype=np.float32)
    for i in range(N_CORES):
        r0 = i * ROWS_PER_CORE
        keep = min(ROWS_PER_CORE, OUT_H - r0)
        out[r0:r0 + keep] = res.results[i]["y"][:keep]
    return out


# revision 5
# speedup vs baseline: 1.0870x; 1.0870x over previous
"""Trainium2 Bass kernel: 3x3 valid 2D cross-correlation on an 8192x8192 f32 image.

Strategy (8 NeuronCores, pure spatial/data parallel):
  - Row-shard: core i receives input rows [1024*i, 1024*i + 1026) (the 2-row
    halo comes for free since we shard on the host from the full input; the
    tail cores' out-of-range rows are zero-padded and the corresponding
    output rows discarded at gather).
  - Per core: for each 128-input-row tile, the conv is computed as 3
    TensorEngine matmuls accumulating into PSUM:
        out[y, c] = sum_dx (M_dx.T @ X)[y, c+dx]
    where M_dx[k, y] = w[k-y, dx] is a 3-diagonal band matrix built on the
    host from the 3x3 weight. Data moves through the PE in float32r mode
    (full fp32 bits, fast 4-byte streaming path, ~1 cycle/column).
  - PSUM chunks (512 cols) are evacuated to SBUF by the Vector/Scalar
    engines (alternating), then DMA'd to DRAM.
"""

import numpy as np

import concourse.mybir as mybir
from concourse import bacc
from concourse.tile import TileContext
from concourse.bass_utils import run_bass_kernel_spmd

H = W = 8192
KH = KW = 3
N_CORES = 8
OUT_H = H - KH + 1  # 8190
OUT_W = W - KW + 1  # 8190

ROWS_PER_CORE = 1024          # output rows computed per core (core 7: keep 1022)
IN_ROWS_PER_CORE = ROWS_PER_CORE + KH - 1  # 1026
TILE_OUT = 126                # output rows per 128-partition input tile
CHUNK = 512                   # PSUM bank width (fp32)

_NC_CACHE = {}


def _build_program():
    """Build the per-core Bass program (identical on all 8 cores)."""
    nc = bacc.Bacc("TRN2", target_bir_lowering=False, debug=False)
    x = nc.declare_dram_parameter(
        "x", [IN_ROWS_PER_CORE, W], mybir.dt.float32r, isOutput=False
    )
    m = nc.declare_dram_parameter(
        "m", [128, 3 * TILE_OUT], mybir.dt.float32r, isOutput=False
    )
    y = nc.declare_dram_parameter(
        "y", [ROWS_PER_CORE, OUT_W], mybir.dt.float32, isOutput=True
    )

    n_tiles = -(-ROWS_PER_CORE // TILE_OUT)  # 9 (8 full + 1 of 16 rows)
    n_chunks = -(-OUT_W // CHUNK)            # 16 (15 full + 1 of 510)

    HALF_CHUNKS = n_chunks // 2   # 8 chunks per output half-tile
    HALF_W = HALF_CHUNKS * CHUNK  # 4096

    with TileContext(nc) as tc:
        with (
            tc.tile_pool(name="mp", bufs=1) as mpool,
            tc.tile_pool(name="xp", bufs=3) as xpool,
            tc.tile_pool(name="op", bufs=4) as opool,
            tc.tile_pool(name="pp", bufs=8, space="PSUM") as ppool,
        ):
            # first x tile load leads the program so DMA starts immediately
            xts = [None] * n_tiles
            xts[0] = xpool.tile([128, W], mybir.dt.float32r, name="xt", tag="xt")
            nc.sync.dma_start(out=xts[0][:128], in_=x[0:128, :])

            mt = mpool.tile([128, 3 * TILE_OUT], mybir.dt.float32r)
            nc.sync.dma_start(out=mt[:], in_=m[:])

            for t in range(n_tiles):
                r0 = t * TILE_OUT
                rows_out = min(TILE_OUT, ROWS_PER_CORE - r0)
                rows_in = rows_out + KH - 1

                if xts[t] is None:
                    xts[t] = xpool.tile([128, W], mybir.dt.float32r, name="xt", tag="xt")
                    nc.sync.dma_start(out=xts[t][:rows_in],
                                      in_=x[r0:r0 + rows_in, :])
                xt = xts[t]

                for h in range(2):
                    h0 = h * HALF_W
                    hw = min(HALF_W, OUT_W - h0)
                    ot = opool.tile([128, HALF_W], mybir.dt.float32,
                                    name="ot", tag="ot")
                    for ci in range(HALF_CHUNKS):
                        c0 = h0 + ci * CHUNK
                        wid = min(CHUNK, OUT_W - c0)
                        pt = ppool.tile([128, CHUNK], mybir.dt.float32,
                                        name="pt", tag="pt")
                        for dx in range(KW):
                            nc.tensor.matmul(
                                pt[:rows_out, :wid],
                                mt[:rows_in, dx * TILE_OUT:dx * TILE_OUT + rows_out],
                                xt[:rows_in, c0 + dx:c0 + dx + wid],
                                start=(dx == 0),
                                stop=(dx == KW - 1),
                            )
                        dst = ot[:rows_out, ci * CHUNK:ci * CHUNK + wid]
                        if ci % 2 == 0:
                            nc.scalar.copy(out=dst, in_=pt[:rows_out, :wid])
                        else:
                            nc.vector.tensor_copy(out=dst, in_=pt[:rows_out, :wid])
                    nc.scalar.dma_start(out=y[r0:r0 + rows_out, h0:h0 + hw],
                                        in_=ot[:rows_out, :hw])
    nc.compile()
    return nc


def _get_program():
    if "nc" not in _NC_CACHE:
        _NC_CACHE["nc"] = _build_program()
    return _NC_CACHE["nc"]


def _band_matrices(weight: np.ndarray) -> np.ndarray:
    """m[k, dx*126 + y] = w[k-y, dx] for 0 <= k-y < 3."""
    m = np.zeros((128, 3 * TILE_OUT), dtype=np.float32)
    for dx in range(KW):
        for dy in range(KH):
            ys = np.arange(TILE_OUT)
            m[ys + dy, dx * TILE_OUT + ys] = weight[dy, dx]
    return m


def kernel(x: np.ndarray, weight: np.ndarray) -> np.ndarray:
    x = np.ascontiguousarray(np.asarray(x, dtype=np.float32))
    weight = np.asarray(weight, dtype=np.float32)
    assert x.shape == (H, W) and weight.shape == (KH, KW)

    m = _band_matrices(weight)

    # shard rows with halo; zero-pad past the bottom edge
    in_maps = []
    for i in range(N_CORES):
        r0 = i * ROWS_PER_CORE
        r1 = min(r0 + IN_ROWS_PER_CORE, H)
        shard = np.zeros((IN_ROWS_PER_CORE, W), dtype=np.float32)
        shard[: r1 - r0] = x[r0:r1]
        in_maps.append({"x": shard, "m": m})

    nc = _get_program()
    res = run_bass_kernel_spmd(nc, in_maps, core_ids=list(range(N_CORES)))

    out = np.empty((OUT_H, OUT_W), dtype=np.float32)
    for i in range(N_CORES):
        r0 = i * ROWS_PER_CORE
        keep = min(ROWS_PER_CORE, OUT_H - r0)
        out[r0:r0 + keep] = res.results[i]["y"][:keep]
    return out
